# revision 109
# baseline (speedup 1.0000x reference)
"""Trainium2 Bass kernel for batched causal multi-head attention.

Problem: x[B=8,S=1024,D=768], per-head projections W_Q/W_K/W_V [H=12,D,DH=64],
W_O [H,DH,D]; causal softmax attention; output [B,S,D].

Strategy: data-parallel over batch across 8 NeuronCores (no collectives).
Per core (one batch element), computed fully on-chip:
  - All four projections run fp8-e4m3 with DoubleRow (2 K-planes per PE cell,
    ~2x matmul rate; host pre-scales x by 16 and weights by 64, the 1/1024
    folds into the PSUM-evacuation copies). Precision carve-outs keep bf16
    where fp8 noise would concentrate: V-projection s-tile 0 (the only keys
    early queries attend to) and O-projection s-tile 0 (early rows' z spans
    few keys). The z-matmul likewise runs fp8 DoubleRow over key-tile PAIRS
    (v stored fp8 x16 padded to 68-col head groups for the 16B-aligned Ko
    stride; softmax weights quantize to fp8 post-exp) except the ib0/jt0 step,
    which stays bf16. Q/K projections and scores tolerate fp8 everywhere
    (logit noise is divided by sqrt(DH) and absorbed by softmax; measured
    max-rel-err 1.03e-2 vs 3.8e-3 all-bf16, threshold 2e-2).
  - scores^T [j, i] = kT.T @ qT per head pair with causal block-skipping; exp
    on ScalarE (all scale factors folded); diagonal-block mask via a 0/1 mask
    multiply; the two heads sit on PE row-tiles (0,0)/(64,0) so hardware may
    overlap them. v carries a ones column so the z-matmul also produces the
    softmax denominator (row 64).
  - Normalization of each (pair, i-block): one den DMA to 32-aligned
    partitions, per-block reciprocal, selector-matmul broadcast (rows 0-63 <-
    den of even head, 64-127 <- odd), one DVE multiply writing fp8 z. The
    chain is emitted one block LATE so the in-order PE queue never waits on
    it; two rotating den tiles bound the deferral to one block.
  - Emission schedule interleaves DVE-bound ib0 blocks with ACT(exp)-bound
    ib1 blocks of the previous pair, hoists pair 0 into the V-projection
    window, and hides the final chains behind out-projections. PSUM: shared
    [128,512]f32 tag (projections + out-proj, 2 bufs), scores [128,2,512]
    (2 bufs), z/selector [65|128,512] (2 bufs) = 8 banks.
  - DMA count is minimized (every DMA serializes ~625ns through HWDGE); input
    images are packed host-side (free) into exact SBUF layouts.
`reps`/`loop_reps` are benchmarking aids (static unroll / on-device For_i).
Simulated (TimelineSim) 103.1us vs 152.1us for the all-bf16 baseline; the
baseline's graded HW time was 149.4us and matched the simulator within 2%.
"""

import os
from contextlib import ExitStack

import numpy as np

B, S, D, H, DH = 8, 1024, 768, 12, 64
P = 128
DT = 6  # d tiles (D / 128)
ST = 8  # s tiles (S / 128)
PAIRS = 6  # head pairs (H / 2)
NB = 512  # i-block width
SCALE = 1.0 / 8.0  # 1/sqrt(DH)

_CACHE = {}


def _build(qk_bias: bool, v_bias: bool, reps: int = 1, loop_reps: int = 0):
    import concourse.bass as bass  # noqa: F401
    import concourse.mybir as mybir
    import concourse.tile as tile
    from concourse import bacc

    f32 = mybir.dt.float32
    bf16 = mybir.dt.bfloat16
    Exp = mybir.ActivationFunctionType.Exp

    nc = bacc.Bacc("TRN2", target_bir_lowering=False, debug=False)

    f8 = mybir.dt.float8e4
    DR = mybir.MatmulPerfMode.DoubleRow
    xT = nc.dram_tensor("xT", [P, DT, P], bf16, kind="ExternalInput").ap()
    xf = nc.dram_tensor("xf", [P, DT, S], f8, kind="ExternalInput").ap()
    wq = nc.dram_tensor("wq", [P, PAIRS, DT, P], f8, kind="ExternalInput").ap()
    wk = nc.dram_tensor("wk", [P, PAIRS, DT, P], f8, kind="ExternalInput").ap()
    wv = nc.dram_tensor("wv", [P, DT, D], bf16, kind="ExternalInput").ap()
    wvf = nc.dram_tensor("wvf", [P, DT, D], f8, kind="ExternalInput").ap()
    wo = nc.dram_tensor("wo", [P, PAIRS, D], bf16, kind="ExternalInput").ap()
    wof = nc.dram_tensor("wof", [P, PAIRS, D], f8, kind="ExternalInput").ap()
    mask2 = nc.dram_tensor("mask2", [P, 2, P], bf16, kind="ExternalInput").ap()
    if qk_bias:
        bq = nc.dram_tensor("bq", [P, PAIRS], f32, kind="ExternalInput").ap()
        bk = nc.dram_tensor("bk", [P, PAIRS], f32, kind="ExternalInput").ap()
    if v_bias:
        bv = nc.dram_tensor("bv", [1, D], f32, kind="ExternalInput").ap()
    out = nc.dram_tensor("out", [S, D], bf16, kind="ExternalOutput").ap()

    def mmr(o, lhsT, rhs, start, stop):
        nc.tensor.matmul(o, lhsT, rhs, start=start, stop=stop)

    with tile.TileContext(nc) as tc:
      with ExitStack() as loop_ctx:
        if loop_reps:
            loop_ctx.enter_context(tc.For_i(0, loop_reps, 1))
        for _rep in range(reps):
          with ExitStack() as ctx:
            consts = ctx.enter_context(tc.tile_pool(name="consts", bufs=1))
            xt_p = ctx.enter_context(tc.tile_pool(name="xt", bufs=1))
            w_p = ctx.enter_context(tc.tile_pool(name="w", bufs=1))
            v_p = ctx.enter_context(tc.tile_pool(name="v", bufs=1))
            z_p = ctx.enter_context(tc.tile_pool(name="z", bufs=1))
            qk_p = ctx.enter_context(tc.tile_pool(name="qk", bufs=1))
            p_p = ctx.enter_context(tc.tile_pool(name="p", bufs=6))
            rec_p = ctx.enter_context(tc.tile_pool(name="rec", bufs=8))
            out_p = ctx.enter_context(tc.tile_pool(name="out", bufs=6))

            # DMA order + chunking: xt/wv gate the first v-proj matmuls, so
            # land them in fine-grained pieces (Tile deps are AP-range aware);
            # wq/wk per pair; wo/mask are needed much later.
            # Every DMA serializes ~625ns through the global HWDGE, so DMA
            # count is the scarce resource: one DMA per x s-tile (contiguous
            # 1.5KB/partition in this layout), wv per dt (gates the first
            # matmuls), everything else whole.
            xt = xt_p.tile([P, DT, P], bf16)
            wv_t = w_p.tile([P, DT, D], bf16, tag="wv")
            xf_t = xt_p.tile([P, DT, S], f8, tag="xf")
            wvf_t = w_p.tile([P, DT, D], f8, tag="wvf")
            # phase A starts on the fp8 s-tiles: chunk wvf/xf by dt-pair so
            # the first matmul waits on only two small DMAs
            for c in range(DT // 2):
                nc.sync.dma_start(
                    out=wvf_t[:, 2 * c : 2 * c + 2, :],
                    in_=wvf[:, 2 * c : 2 * c + 2, :],
                )
                nc.sync.dma_start(
                    out=xf_t[:, 2 * c : 2 * c + 2, :],
                    in_=xf[:, 2 * c : 2 * c + 2, :],
                )
            wq_t = w_p.tile([P, PAIRS, DT, P], f8, tag="wq")
            wk_t = w_p.tile([P, PAIRS, DT, P], f8, tag="wk")
            nc.sync.dma_start(
                out=wq_t[:, 0:1, :, :], in_=wq[:, 0:1, :, :]
            )
            nc.sync.dma_start(
                out=wk_t[:, 0:1, :, :], in_=wk[:, 0:1, :, :]
            )
            nc.sync.dma_start(out=xt[:, :, :], in_=xT[:, :, :])
            for dt in range(DT):
                nc.sync.dma_start(
                    out=wv_t[:, dt : dt + 1, :], in_=wv[:, dt : dt + 1, :]
                )
            nc.sync.dma_start(out=wq_t[:, 1:, :, :], in_=wq[:, 1:, :, :])
            nc.sync.dma_start(out=wk_t[:, 1:, :, :], in_=wk[:, 1:, :, :])
            mask2_t = consts.tile([P, 2, P], bf16)
            nc.sync.dma_start(out=mask2_t[:, :, :], in_=mask2[:, :, :])
            wo_t = w_p.tile([P, PAIRS, D], bf16, tag="wo")
            nc.sync.dma_start(out=wo_t[:, :, :], in_=wo[:, :, :])
            wof_t = w_p.tile([P, PAIRS, D], f8, tag="wof")
            nc.sync.dma_start(out=wof_t[:, :, :], in_=wof[:, :, :])
            if qk_bias:
                bq_t = consts.tile([P, PAIRS], f32, tag="bq")
                nc.sync.dma_start(out=bq_t[:, :], in_=bq[:, :])
                bk_t = consts.tile([P, PAIRS], f32, tag="bk")
                nc.sync.dma_start(out=bk_t[:, :], in_=bk[:, :])
            if v_bias:
                bv_row = consts.tile([P, D], f32, tag="bvr")
                nc.sync.dma_start(out=bv_row[0:1, :], in_=bv[:, :])
                bv_full = consts.tile([P, D], f32, tag="bvf")
                nc.gpsimd.partition_broadcast(bv_full[:, :], bv_row[0:1, :])
                bv16 = consts.tile([P, D], f32, tag="bv16")
                nc.vector.tensor_scalar_mul(bv16[:, :], bv_full[:, :], 16.0)

            # v layout: [s-tile, head, 68] — fp8, values pre-scaled by 16,
            # head group padded to 68 so the s-tile stride is 16B-aligned for
            # DoubleRow; col 64 of each head group is 1.0 (the ones column
            # makes the z-matmul also produce the softmax denominator).
            # v0_t is an exact bf16 copy of s-tile 0 (the only keys early
            # queries see).
            VP = DH + 4
            v_t = v_p.tile([P, ST, H, VP], f8)
            nc.vector.memset(v_t[:, :, :, DH], 1.0)
            v0_t = v_p.tile([P, H, DH + 1], bf16, tag="v0")
            nc.vector.memset(v0_t[:, :, DH], 1.0)

            z_t = z_p.tile([P, PAIRS, S], bf16)
            zf_t = z_p.tile([P, PAIRS, S], f8, tag="zf")
            z0_t = z_p.tile([P, PAIRS, P], bf16, tag="z0")
            # per-(pair, ib) denominators: head-even at partition 0, head-odd
            # at partition 32 (DMA start partitions must be 32-aligned). Two
            # persistent tiles alternate across g-slots; untouched rows stay
            # 1.0 from this one-time memset so the reciprocal is finite.
            den_ts = [
                consts.tile([33, NB], bf16, tag=f"den{i}", name=f"den{i}")
                for i in range(2)
            ]
            for dt_ in den_ts:
                nc.vector.memset(dt_[:, :], 1.0)
            # selector: out rows 0-63 <- rec row 0, rows 64-127 <- rec row 32
            sel2 = consts.tile([33, P], f32, tag="sel2")
            nc.vector.memset(sel2[:, :], 0.0)
            nc.vector.memset(sel2[0:1, 0:64], 1.0)
            nc.vector.memset(sel2[32:33, 64:128], 1.0)

            qT_all = qk_p.tile([P, PAIRS, S], bf16, tag="qT")
            kT_all = qk_p.tile([P, PAIRS, S], bf16, tag="kT")

            with (
                tc.tile_pool(name="ps_big", bufs=2, space="PSUM") as ps_big,
                tc.tile_pool(name="ps_sc", bufs=2, space="PSUM") as ps_sc,
                tc.tile_pool(name="ps_z", bufs=2, space="PSUM") as ps_z,
            ):
                # ---------------- Phase A: V projection (all heads) ----------
                def vproj(st):
                    vp1 = ps_big.tile([P, NB], f32, tag="big", name="vp1")
                    vp2 = ps_big.tile([P, NB], f32, tag="big", name="vp2")
                    if st == 0:
                        # bf16: rows 0-127 are the only keys early queries see
                        for dt in range(DT):
                            lhsT = xt[:, dt, :]
                            mmr(
                                vp1[:, :], lhsT, wv_t[:, dt, 0:NB],
                                dt == 0, dt == DT - 1,
                            )
                        for dt in range(DT):
                            lhsT = xt[:, dt, :]
                            mmr(
                                vp2[:, 0 : D - NB], lhsT, wv_t[:, dt, NB:D],
                                dt == 0, dt == DT - 1,
                            )
                        vscale = 16.0
                    else:
                        for c in range(DT // 2):
                            nc.tensor.matmul(
                                vp1[:, :],
                                xf_t[:, 2 * c : 2 * c + 2,
                                     st * P : (st + 1) * P],
                                wvf_t[:, 2 * c : 2 * c + 2, 0:NB],
                                start=c == 0, stop=c == DT // 2 - 1,
                                perf_mode=DR,
                            )
                        for c in range(DT // 2):
                            nc.tensor.matmul(
                                vp2[:, 0 : D - NB],
                                xf_t[:, 2 * c : 2 * c + 2,
                                     st * P : (st + 1) * P],
                                wvf_t[:, 2 * c : 2 * c + 2, NB:D],
                                start=c == 0, stop=c == DT // 2 - 1,
                                perf_mode=DR,
                            )
                        vscale = 16.0 / 1024.0
                    nc.vector.tensor_scalar_mul(
                        v_t[:, st, 0:8, 0:DH],
                        vp1.rearrange("p (h e) -> p h e", e=DH),
                        vscale,
                    )
                    nc.vector.tensor_scalar_mul(
                        v_t[:, st, 8:12, 0:DH],
                        vp2[:, 0 : D - NB].rearrange("p (h e) -> p h e", e=DH),
                        vscale,
                    )
                    if st == 0:
                        nc.scalar.mul(
                            v0_t[:, 0:8, 0:DH],
                            vp1.rearrange("p (h e) -> p h e", e=DH),
                            16.0,
                        )
                        nc.scalar.mul(
                            v0_t[:, 8:12, 0:DH],
                            vp2[:, 0 : D - NB].rearrange("p (h e) -> p h e", e=DH),
                            16.0,
                        )
                    if v_bias:
                        nc.vector.tensor_add(
                            v_t[:, st, :, 0:DH],
                            v_t[:, st, :, 0:DH],
                            bv16.rearrange("p (h e) -> p h e", e=DH),
                        )
                        if st == 0:
                            nc.vector.tensor_add(
                                v0_t[:, :, 0:DH],
                                v0_t[:, :, 0:DH],
                                bv16.rearrange("p (h e) -> p h e", e=DH),
                            )

                # ---------------- attention passes --------------------------
                # normalization: den DMA is emitted inline (frees the z PSUM
                # slot); the reciprocal/broadcast/multiply chain is emitted one
                # pair LATE so the in-order PE queue never waits on it.
                pending = []
                blk_seq = [0]

                def emit_norm():
                    pr, ib, seq = pending.pop(0)
                    den_t = den_ts[seq % 2]
                    den_f = rec_p.tile([33, NB], f32, tag="denf")
                    nc.vector.tensor_copy(den_f[:, :], den_t[:, :])
                    rec_t = rec_p.tile([33, NB], f32, tag="rec")
                    nc.vector.reciprocal_approx_fast(rec_t[:, :], den_f[:, :])
                    # bc shares the z-pool tag (same per-partition byte size)
                    bc = ps_z.tile([P, NB], f32, tag="z", name="bc")
                    nc.tensor.matmul(
                        bc[:, :], sel2[:, :], rec_t[:, :], start=True, stop=True
                    )
                    nc.vector.tensor_mul(
                        zf_t[:, pr, ib * NB : (ib + 1) * NB],
                        z_t[:, pr, ib * NB : (ib + 1) * NB],
                        bc[:, :],
                    )
                    if ib == 0:
                        nc.vector.tensor_mul(
                            z0_t[:, pr, :],
                            z_t[:, pr, 0:P],
                            bc[:, 0:P],
                        )

                def attn_block(pr, ib):
                    # scores^T + z^T for queries [ib*NB, (ib+1)*NB), pair pr.
                    # j-tiles are processed in steps: ib0 keeps jt0 in bf16
                    # (early queries' z is dominated by those keys), jt1 runs
                    # fp8 solo, and all later tiles run as fp8 DoubleRow
                    # jt-PAIRS (two key tiles per matmul at fp8 double rate).
                    njt = 4 * (ib + 1)
                    qT_t = qT_all[:, pr, :]
                    kT_t = kT_all[:, pr, :]
                    zps = [
                        ps_z.tile([DH + 1, NB], f32, tag="z", name="zpsA"),
                        ps_z.tile([DH + 1, NB], f32, tag="z", name="zpsB"),
                    ]
                    if ib == 0:
                        steps = [("bf", [0]), ("f8", [1]), ("dr", [2, 3])]
                    else:
                        steps = [("dr", [0, 1]), ("dr", [2, 3]),
                                 ("dr", [4, 5]), ("dr", [6, 7])]

                    def emit_z(kind, jts, pt, os_):
                        o = os_[0]
                        first = jts[0] == 0
                        last = jts[-1] == njt - 1
                        for h2 in range(2):
                            h = 2 * pr + h2
                            if kind == "bf":
                                mmr(
                                    zps[h2][:, :],
                                    v0_t[:, h, :],
                                    pt[:, h2, :],
                                    first, last,
                                )
                            elif kind == "f8":
                                mmr(
                                    zps[h2][:, o:NB],
                                    v_t[:, jts[0], h, 0 : DH + 1],
                                    pt[:, h2, o:NB],
                                    first, last,
                                )
                            else:
                                nc.tensor.matmul(
                                    zps[h2][:, o:NB],
                                    v_t[:, jts[0] : jts[0] + 2, h, 0 : DH + 1],
                                    pt[:, :, h2, o:NB],
                                    start=first, stop=last,
                                    perf_mode=DR,
                                )

                    # staggered: the z-matmul for step s-1 is emitted after
                    # the scores matmuls of step s, so the in-order PE never
                    # stalls on the exp+mask latency of the current step.
                    zq = []
                    for kind, jts in steps:
                        os_ = [max(0, P * jt - NB * ib) for jt in jts]
                        if kind == "bf":
                            pt = p_p.tile([P, 2, NB], bf16, tag="pb", name="ptb")
                        elif kind == "f8":
                            pt = p_p.tile([P, 2, NB], f8, tag="pf1", name="ptf1")
                        else:
                            pt = p_p.tile(
                                [P, 2, 2, NB], f8, tag="pf2", name="ptf2"
                            )
                            if os_[1] > os_[0]:
                                # odd tile's columns below its causal start
                                # feed the DoubleRow matmul: zero them
                                nc.vector.memset(
                                    pt[:, 1, :, os_[0] : os_[1]], 0.0
                                )
                        for idx, jt in enumerate(jts):
                            o = os_[idx]
                            sps = ps_sc.tile([P, 2, NB], f32, tag="sc")
                            for h2 in range(2):
                                mmr(
                                    sps[:, h2, o:NB],
                                    kT_t[64 * h2 : 64 * (h2 + 1), jt * P : (jt + 1) * P],
                                    qT_t[64 * h2 : 64 * (h2 + 1), ib * NB + o : (ib + 1) * NB],
                                    True,
                                    True,
                                )
                            pdst = pt[:, idx, :, :] if kind == "dr" else pt
                            nc.scalar.activation(
                                pdst[:, :, o:NB], sps[:, :, o:NB], Exp,
                                scale=SCALE,
                            )
                            if P * jt - NB * ib >= 0:  # diagonal crossing
                                nc.vector.tensor_mul(
                                    pdst[:, :, o : o + P],
                                    pdst[:, :, o : o + P],
                                    mask2_t[:, :, :],
                                )
                        zq.append((kind, jts, pt, os_))
                        if len(zq) > 1:
                            emit_z(*zq.pop(0))
                    while zq:
                        emit_z(*zq.pop(0))
                    seq = blk_seq[0]
                    blk_seq[0] += 1
                    den_t = den_ts[seq % 2]
                    # both heads' unnormalized z (+ den in row 64, bf16 —
                    # harmless for the denominator) leave PSUM into one tile
                    # (ACT for h2=0, DVE for h2=1 — neither queue gates both),
                    # then ONE z DMA and ONE den DMA place them (den must land
                    # on 32-aligned partitions for the selector matmul).
                    ztmp = rec_p.tile([DH + 1, 2, NB], bf16, tag="ztmp")
                    nc.vector.tensor_copy(ztmp[:, 0, :], zps[0][:, :])
                    nc.vector.tensor_copy(ztmp[:, 1, :], zps[1][:, :])
                    nc.sync.dma_start(
                        den_t[0:33:32, :],
                        ztmp[DH : DH + 1, :, :],
                    )
                    for h2 in range(2):
                        nc.sync.dma_start(
                            z_t[64 * h2 : 64 * (h2 + 1), pr, ib * NB : (ib + 1) * NB],
                            ztmp[0:DH, h2, :],
                        )
                    pending.append((pr, ib, seq))

                def out_proj(st):
                    op1 = ps_big.tile([P, NB], f32, tag="big", name="op1")
                    op2 = ps_big.tile([P, NB], f32, tag="big", name="op2")
                    if st == 0:
                        # bf16 path: early rows' z spans few keys, fp8 noise
                        # there would dominate; z0 carries 16*z_norm
                        for pr in range(PAIRS):
                            lhsT = z0_t[:, pr, :]
                            mmr(
                                op1[:, :], lhsT, wo_t[:, pr, 0:NB],
                                pr == 0, pr == PAIRS - 1,
                            )
                        for pr in range(PAIRS):
                            lhsT = z0_t[:, pr, :]
                            mmr(
                                op2[:, 0 : D - NB], lhsT, wo_t[:, pr, NB:D],
                                pr == 0, pr == PAIRS - 1,
                            )
                        oscale = 1.0 / 16.0
                    else:
                        for c in range(PAIRS // 2):
                            nc.tensor.matmul(
                                op1[:, :],
                                zf_t[:, 2 * c : 2 * c + 2,
                                     st * P : (st + 1) * P],
                                wof_t[:, 2 * c : 2 * c + 2, 0:NB],
                                start=c == 0, stop=c == PAIRS // 2 - 1,
                                perf_mode=DR,
                            )
                        for c in range(PAIRS // 2):
                            nc.tensor.matmul(
                                op2[:, 0 : D - NB],
                                zf_t[:, 2 * c : 2 * c + 2,
                                     st * P : (st + 1) * P],
                                wof_t[:, 2 * c : 2 * c + 2, NB:D],
                                start=c == 0, stop=c == PAIRS // 2 - 1,
                                perf_mode=DR,
                            )
                        oscale = 1.0 / 1024.0
                    # both copies on ACT (DVE's strict FIFO may be blocked by a
                    # deferred z-normalize multiply); out DMA takes the ACT
                    # HWDGE queue, away from the z/den DMAs on SP
                    ot = out_p.tile([P, D], bf16, tag="ot")
                    nc.scalar.mul(ot[:, 0:NB], op1[:, :], oscale)
                    nc.vector.tensor_scalar_mul(
                        ot[:, NB:D], op2[:, 0 : D - NB], oscale
                    )
                    nc.sync.dma_start(out[st * P : (st + 1) * P, :], ot[:, :])

                def qkproj(pr):
                    # ib0 halves first: they gate the pair's first score
                    # matmuls, the ib1 halves are not needed until a pair later
                    for ib in range(2):
                        for dst, w_t, b_t in (
                            (qT_all[:, pr, :], wq_t, "bq"),
                            (kT_all[:, pr, :], wk_t, "bk"),
                        ):
                            ps = ps_big.tile([P, NB], f32, tag="big", name="qk")
                            for c in range(DT // 2):
                                nc.tensor.matmul(
                                    ps[:, :],
                                    w_t[:, pr, 2 * c : 2 * c + 2, :],
                                    xf_t[:, 2 * c : 2 * c + 2,
                                         ib * NB : (ib + 1) * NB],
                                    start=c == 0,
                                    stop=c == DT // 2 - 1,
                                    perf_mode=DR,
                                )
                            # evacuate with the fp8 pre-scale (x*16, w*64)
                            # divided back out; alternate engines so neither
                            # ACT nor DVE queueing gates the next PSUM reuse
                            if b_t == "bq" and ib == 0:
                                nc.scalar.mul(
                                    dst[:, ib * NB : (ib + 1) * NB],
                                    ps[:, :],
                                    1.0 / 1024.0,
                                )
                            else:
                                nc.vector.tensor_scalar_mul(
                                    dst[:, ib * NB : (ib + 1) * NB],
                                    ps[:, :],
                                    1.0 / 1024.0,
                                )
                            if qk_bias and ib == 1:
                                bias_ap = (bq_t if b_t == "bq" else bk_t)[
                                    :, pr : pr + 1
                                ]
                                nc.vector.tensor_scalar_add(
                                    dst[:, :], dst[:, :], bias_ap
                                )
                # ---- emission schedule: v-proj s-tiles 1-3, then pair 0's
                # projections and s-tile 0 (bf16), then pair 0's first block —
                # exps start while the PE still works v-proj s-tiles 4-7.
                # Then one merged loop; ib=0 steps are DVE-bound, ib=1 steps
                # ACT(exp)-bound, so alternating them averages both engines.
                for st in (1, 2, 3):
                    vproj(st)
                qkproj(0)
                vproj(0)
                vproj(4)
                attn_block(0, 0)
                vproj(5)
                qkproj(1)
                vproj(6)
                vproj(7)
                for pr in range(1, PAIRS):
                    if pr >= 2:
                        qkproj(pr)
                    attn_block(pr, 0)
                    if len(pending) > 1:
                        # norms must drain after each block: with only two
                        # alternating den tiles, a norm deferred two blocks
                        # would read a denominator already overwritten
                        emit_norm()
                    # interleave the previous pair's ib=1 block: ib=0 steps
                    # are DVE-bound, ib=1 steps ACT(exp)-bound, so
                    # alternating them averages both engines' load
                    attn_block(pr - 1, 1)
                    if len(pending) > 1:
                        emit_norm()

                # tail: last ib=1 block, then the final normalization chains
                # hidden behind the (pass-0-data only) out-projections of
                # s-tiles 0-2; s-tiles 3-7 close the kernel
                attn_block(PAIRS - 1, 1)
                emit_norm()
                out_proj(0)
                out_proj(1)
                out_proj(2)
                while pending:
                    emit_norm()
                out_proj(3)
                for st in range(4, ST):
                    out_proj(st)

    nc.compile()
    return nc


def _pack_host(inputs):
    import ml_dtypes

    bf = ml_dtypes.bfloat16
    x = np.ascontiguousarray(np.asarray(inputs["normalized_resid_pre"], np.float32))
    WQ = np.asarray(inputs["W_Q"], np.float32)
    WK = np.asarray(inputs["W_K"], np.float32)
    WV = np.asarray(inputs["W_V"], np.float32)
    WO = np.asarray(inputs["W_O"], np.float32)

    f8 = ml_dtypes.float8_e4m3

    def pack_qk(W):
        img = np.empty((P, PAIRS, DT, P), np.float32)
        for pr in range(PAIRS):
            for dt in range(DT):
                img[:, pr, dt, 0:64] = W[2 * pr, dt * P : (dt + 1) * P, :]
                img[:, pr, dt, 64:128] = W[2 * pr + 1, dt * P : (dt + 1) * P, :]
        return np.ascontiguousarray(img)

    def to_f8(a, scale):
        return np.clip(a * scale, -240.0, 240.0).astype(f8)

    wq_img = to_f8(pack_qk(WQ), 64.0)
    wk_img = to_f8(pack_qk(WK), 64.0)
    # wv_sb[p, dt, n] = WV_flat[dt*128+p, n];  WV_flat[d, h*64+e] = WV[h, d, e]
    wv_flat = WV.transpose(1, 0, 2).reshape(D, D)
    wv_pack = np.ascontiguousarray(wv_flat.reshape(DT, P, D).transpose(1, 0, 2))
    wv_img = wv_pack.astype(bf)
    wvf_img = to_f8(wv_pack, 64.0)
    # wo_sb[p, pr, n]: rows stack the pair's two heads' DH dims
    wo_pack = np.ascontiguousarray(WO.reshape(PAIRS, P, D).transpose(1, 0, 2))
    wo_img = wo_pack.astype(bf)
    wof_img = to_f8(wo_pack, 64.0)
    m = (np.arange(P)[:, None] <= np.arange(P)[None, :]).astype(np.float32)
    mask2_img = np.ascontiguousarray(np.stack([m, m], axis=1)).astype(bf)
    # xT_img[p, st, dt, c] = x[st*128 + c, dt*128 + p]
    xT_imgs = [
        np.ascontiguousarray(
            x[b, 0:P].T.reshape(DT, P, P).transpose(1, 0, 2)
        ).astype(bf)
        for b in range(B)
    ]
    xf_imgs = [
        to_f8(
            np.ascontiguousarray(x[b].T.reshape(DT, P, S).transpose(1, 0, 2)),
            16.0,
        )
        for b in range(B)
    ]
    return (xT_imgs, xf_imgs, wq_img, wk_img, wv_img, wvf_img, wo_img,
            wof_img, mask2_img)


def _make_in_maps(packed, inputs, qk_bias, v_bias):
    (xT_imgs, xf_imgs, wq_img, wk_img, wv_img, wvf_img, wo_img, wof_img,
     mask2_img) = packed
    common = {
        "wq": wq_img,
        "wk": wk_img,
        "wv": wv_img,
        "wvf": wvf_img,
        "wo": wo_img,
        "wof": wof_img,
        "mask2": mask2_img,
    }
    if qk_bias:
        bq_np = np.asarray(inputs["b_Q"], np.float32)
        bk_np = np.asarray(inputs["b_K"], np.float32)
        common["bq"] = np.ascontiguousarray(bq_np.reshape(PAIRS, P).T)
        common["bk"] = np.ascontiguousarray(bk_np.reshape(PAIRS, P).T)
    if v_bias:
        bv_np = np.asarray(inputs["b_V"], np.float32)
        common["bv"] = np.ascontiguousarray(bv_np.reshape(1, D))
    return [
        dict(common, xT=xT_imgs[b], xf=xf_imgs[b]) for b in range(B)
    ]


def kernel(**inputs):
    global LAST_EXEC_TIME_NS
    from concourse.bass_utils import run_bass_kernel_spmd

    bq_np = np.asarray(inputs["b_Q"], np.float32)
    bk_np = np.asarray(inputs["b_K"], np.float32)
    bv_np = np.asarray(inputs["b_V"], np.float32)
    bo_np = np.asarray(inputs["b_O"], np.float32)
    qk_bias = bool(np.any(bq_np) or np.any(bk_np))
    v_bias = bool(np.any(bv_np))

    reps = int(os.environ.get("KERNEL_REPS", "1"))
    key = (qk_bias, v_bias, reps)
    if key not in _CACHE:
        _CACHE[key] = _build(qk_bias, v_bias, reps)
    nc = _CACHE[key]

    in_maps = _make_in_maps(_pack_host(inputs), inputs, qk_bias, v_bias)

    trace = os.environ.get("KERNEL_TRACE", "0") == "1"
    try:
        res = run_bass_kernel_spmd(
            nc, in_maps, core_ids=list(range(B)), trace=trace
        )
    except ModuleNotFoundError:
        # axon NTFF profiling hook unavailable in this container
        res = run_bass_kernel_spmd(nc, in_maps, core_ids=list(range(B)))
    LAST_EXEC_TIME_NS = res.exec_time_ns
    if trace and res.exec_time_ns is not None:
        print(f"HW exec time: {res.exec_time_ns} ns")

    out = np.stack(
        [np.asarray(res.results[b]["out"], np.float32) for b in range(B)], axis=0
    )
    out = out + bo_np[None, None, :]
    return out.astype(np.float32)


LAST_EXEC_TIME_NS = None
